# revision 46
# baseline (speedup 1.0000x reference)
"""Causal attention head on 8 trn2 NeuronCores.

Sharding: core c = (batch b = c//2, type t = c%2). Each core handles 8
query stripes of 256 (one slot per stripe) of its batch against the key
prefix each stripe needs. Causal balance: type A gets stripes with nkb
{32,28,24,20,14,10,6,2}, type B {30,26,22,18,16,12,8,4} (136 key-blocks
each). One SPMD program: every core runs the padded template
T8 = {32,28,24,20,16,12,8,4} (144 units); masking and padding are driven
purely by per-core input data (multiplicative bf16 mask tiles), so the
instruction stream is identical on all cores.

Design notes:
  - AV matmul flipped: e (bf16) is the STATIONARY operand, v the moving
    one. v is augmented with a ones column -> rhs [128 keys, 257]; the
    softmax denominator accumulates in psum column 256 for free (no
    separate denominator matmuls; PE matmul cost is proportional to the
    moving-side column count only).
  - Output accumulates as [q, dims] (natural layout). Denominator is a
    per-partition scalar -> reciprocal + broadcast multiply on DVE;
    normalized output DMAs out directly, no host math.
  - Causal early stop per q-chunk: chunk cc of slot j stops its psum
    accumulation at template unit T8[j]-2+cc (beyond-diagonal units are
    masked-zero on every core), then normalizes + ships immediately.
  - Encodings/weights travel as bf16 (quantizing the matmul inputs
    instead of the projection outputs; rel err ~5e-3 vs 3e-3) which
    halves input DMA to ~5.3MB/core.
  - All weights, mask thresholds (clamped to [-1,256], exact in bf16)
    and query stripe 7 ride in one packed input `wa`, so the first
    DMA pair unblocks the whole startup ladder; ek/ev stream in graded
    key chunks (512,1024,1024,1536) matched to slot unlock order.
  - Causal masks are generated on device (gpsimd iota ramp +
    tensor_scalar is_ge against per-core thresholds) instead of DMAed.
  - exp in 2-unit groups ([128,512] activations, 72/core); global lag-4
    software pipeline across slots with kT/qT/v projection tasks
    injected between attention phases to keep PE busy.

Per slot j (query stripe q0..q0+256, tj = T8[j] units):
  sT[u]  = (k[kb=u] @ qT_stripe)      scores transposed [128 keys, 256 q]
  e      = exp(0.125*sT) (*mask)      bf16, groups of 2 units
  po[c] += e[:,c*128:+128].T @ [v[u] | 1]   [128 q, 257] psum, c=0..1
  out[c] = po[c,:256] * (1/po[c,256])       DVE, DMA out [128 q, 256]
"""

import sys

sys.path.insert(0, "/opt/trn_rl_repo")

import numpy as np
import ml_dtypes

B, S, DM, DQ = 4, 4096, 256, 64
T8 = [32, 28, 24, 20, 16, 12, 8, 4]  # padded template: key-blocks per slot
R_A8 = [32, 28, 24, 20, 14, 10, 6, 2]  # real key-blocks, type A cores
R_B8 = [30, 26, 22, 18, 16, 12, 8, 4]  # type B

_CACHE = {}


def _q0s8(R):
    return [(r - 2) * 128 for r in R]


def _build_nc():
    import concourse.bass as bass  # noqa: F401
    import concourse.tile as tile
    from concourse import bacc, mybir

    dt = mybir.dt
    f32, bf = dt.float32, dt.bfloat16
    f32r = dt.float32r

    nc = bacc.Bacc(
        "TRN2",
        target_bir_lowering=False,
        debug=False,
        enable_asserts=False,
        num_devices=8,
    )

    def din(name, shape, d=None):
        d = f32r if d is None else d
        return nc.dram_tensor(name, shape, d, kind="ExternalInput").ap()

    eq = din("eq", [256, 1792], bf)  # stripes 6..0 (stripe 7 rides in wa)
    ek = din("ek", [256, 4096], bf)
    ev = din("ev", [256, 4096], bf)
    # packed [Wq.T | Wk.T | Wv.T | mt | eq stripe 7]; mask thresholds mt
    # (clamped to [-1,256], exact in bf16) live in rows 0:128 of cols
    # 384:416
    wa = din("wa", [256, 672], bf)
    outN = nc.dram_tensor("outN", [2048, 256], f32, kind="ExternalOutput").ap()

    with tile.TileContext(nc) as tc:
        from contextlib import ExitStack

        with ExitStack() as ctx:
            const = ctx.enter_context(tc.tile_pool(name="const", bufs=1))

            # ---- persistent SBUF tensors ----
            eq_sb = [const.tile([128, 1792], bf, tag=f"eq{h}", name=f"eq{h}") for h in range(2)]
            ek_sb = [const.tile([128, 4096], bf, tag=f"ek{h}", name=f"ek{h}") for h in range(2)]
            ev_sb = [const.tile([128, 4096], bf, tag=f"ev{h}", name=f"ev{h}") for h in range(2)]
            wa_sb = const.tile([128, 1344], bf, tag="wa", name="wa")  # 2 x 672

            def wq_s(h):  # [dm_half, 64]
                return wa_sb[:, h * 672 : h * 672 + 64]

            def wk_s(h):
                return wa_sb[:, h * 672 + 64 : h * 672 + 128]

            def wv_s(h):  # [dm_half, 256]
                return wa_sb[:, h * 672 + 128 : h * 672 + 384]

            def eq7_s(h):  # [dm_half, 256] query stripe 7
                return wa_sb[:, h * 672 + 416 : h * 672 + 672]

            mt_sl = wa_sb[:, 384:416]  # thresholds ride in half 0 (bf16)
            mt_f = const.tile([128, 32], dt.float32, tag="mtf", name="mtf")
            m_sb = const.tile([128, 8 * 1024], bf, tag="mi", name="mi")
            cif = const.tile([128, 256], dt.float32, tag="cif", name="cif")
            kT = const.tile([64, 4096], f32r, tag="kT", name="kT")
            qT = const.tile([64, 2048], f32r, tag="qT", name="qT")
            v_sb = const.tile([128, 32 * 257], bf, tag="v", name="v")

            # ones column for every v unit (psum col 256 = softmax denom)
            for t in range(32):
                nc.vector.memset(v_sb[:, t * 257 + 256 : (t + 1) * 257], 1.0)
            # column-index ramp for on-device causal mask generation
            nc.gpsimd.iota(
                cif[:],
                pattern=[[1, 256]],
                base=0,
                channel_multiplier=0,
                allow_small_or_imprecise_dtypes=True,
            )

            # ---- DMAs (SP queue; graded chunks, slot-7-first unlock).
            # With bf16 inputs the stream is issue/DGE-serialization bound,
            # so fewer, consolidated DMAs win.
            def dma_eq(c0, c1):
                for h in range(2):
                    nc.sync.dma_start(
                        eq_sb[h][:, c0:c1], eq[h * 128 : (h + 1) * 128, c0:c1]
                    )

            def dma_kv(t_sb, t_dr, k0, k1, eng=None):
                eng = eng or nc.sync
                for h in range(2):
                    eng.dma_start(
                        t_sb[h][:, k0:k1], t_dr[h * 128 : (h + 1) * 128, k0:k1]
                    )

            for h in range(2):
                nc.sync.dma_start(
                    wa_sb[:, h * 672 : (h + 1) * 672], wa[h * 128 : (h + 1) * 128, :]
                )
            nc.vector.tensor_copy(mt_f[:], mt_sl)  # is_ge needs f32 scalars
            dma_kv(ek_sb, ek, 0, 512)
            dma_kv(ev_sb, ev, 0, 512)
            dma_eq(0, 7 * 256)  # stripes 6..0
            for k0, k1 in ((512, 1536), (1536, 2560), (2560, 4096)):
                dma_kv(ek_sb, ek, k0, k1)
                dma_kv(ev_sb, ev, k0, k1)

            # ---- pools ----
            ps_pool = ctx.enter_context(tc.tile_pool(name="psc", bufs=2, space="PSUM"))
            pp_pool = ctx.enter_context(tc.tile_pool(name="pp", bufs=2, space="PSUM"))
            po_pool = ctx.enter_context(tc.tile_pool(name="po", bufs=2, space="PSUM"))
            epool = ctx.enter_context(tc.tile_pool(name="e", bufs=6))
            opool = ctx.enter_context(tc.tile_pool(name="o", bufs=2))
            rpool = ctx.enter_context(tc.tile_pool(name="r", bufs=2))

            Exp = mybir.ActivationFunctionType.Exp

            # ---- projection task list (emitted piecemeal between phases) --
            def task_qT(s):
                def run():
                    ps = pp_pool.tile([128, 512], f32, tag="pp", name="pp")
                    for h in range(2):
                        nc.tensor.matmul(
                            ps[0:64, 0:256],
                            wq_s(h),
                            eq7_s(h) if s == 7 else eq_sb[h][:, s * 256 : (s + 1) * 256],
                            start=(h == 0),
                            stop=(h == 1),
                        )
                    nc.vector.tensor_copy(
                        qT[:, s * 256 : (s + 1) * 256], ps[0:64, 0:256]
                    )
                return run

            def task_kT(k0):
                def run():
                    ps = pp_pool.tile([128, 512], f32, tag="pp", name="pp")
                    for h in range(2):
                        nc.tensor.matmul(
                            ps[0:64, 0:512],
                            wk_s(h),
                            ek_sb[h][:, k0 : k0 + 512],
                            start=(h == 0),
                            stop=(h == 1),
                        )
                    nc.vector.tensor_copy(kT[:, k0 : k0 + 512], ps[0:64, 0:512])
                return run

            def task_v(u):
                def run():
                    ps = pp_pool.tile([128, 512], f32, tag="pp", name="pp")
                    for h in range(2):
                        nc.tensor.matmul(
                            ps[:, 0:256],
                            ev_sb[h][:, u * 128 : (u + 1) * 128],
                            wv_s(h),
                            start=(h == 0),
                            stop=(h == 1),
                        )
                    nc.vector.tensor_copy(v_sb[:, u * 257 : u * 257 + 256], ps[:, 0:256])
                return run

            def proj_tasks(k0, k1, qstripes):
                ts_ = [task_qT(s) for s in qstripes]
                ts_ += [task_kT(k) for k in range(k0, k1, 512)]
                ts_ += [task_v(u) for u in range(k0 // 128, k1 // 128)]
                return ts_

            # proj work injected into each slot's phase stream, ordered by
            # deadline and placed no earlier than its chunk's DMA arrival
            inject_for = {
                7: proj_tasks(512, 1024, [6]) + proj_tasks(1024, 1536, [5]),
                6: proj_tasks(1536, 2048, [4])
                + proj_tasks(2048, 2560, [3])
                + proj_tasks(2560, 3072, [2]),
                5: proj_tasks(3072, 4096, [1, 0]),
            }

            # ---- global lag-4 attention pipeline ----
            from collections import deque

            pending = deque()  # AV groups not yet emitted
            inject = deque()  # projection tasks to sprinkle between phases

            def pop_inject(n):
                for _ in range(n):
                    if inject:
                        inject.popleft()()

            def emit_S(st):
                j, g = st["j"], st["g"]
                tj = T8[j]
                ps = ps_pool.tile([128, 512], f32, tag="ps", name="ps")
                for k in range(2):
                    u = 2 * g + k
                    nc.tensor.matmul(
                        ps[:, k * 256 : (k + 1) * 256],
                        kT[:, u * 128 : (u + 1) * 128],
                        st["qs"],
                        start=True,
                        stop=True,
                    )
                e = epool.tile([128, 512], bf, tag="e", name="e")
                nc.scalar.activation(e[:], ps[:], Exp, scale=0.125)
                if 2 * g >= tj - 4:  # group inside the masked last-4 window
                    w0 = 2 * g - (tj - 4)  # window offset in units (0 or 2)
                    msl = slice(j * 1024 + w0 * 256, j * 1024 + (w0 + 2) * 256)
                    nc.vector.tensor_mul(e[:], e[:], m_sb[:, msl])
                st["e"] = e

            def finish_chunk(st, cc):
                j, po = st["j"], st["po"]
                o, rinv = st["o"], st["rinv"]
                nc.vector.reciprocal(rinv[:, cc : cc + 1], po[cc][:, 256:257])
                nc.vector.tensor_scalar_mul(
                    o[:, cc * 256 : (cc + 1) * 256],
                    po[cc][:, 0:256],
                    rinv[:, cc : cc + 1],
                )
                r0 = j * 256 + cc * 128
                nc.sync.dma_start(
                    outN[r0 : r0 + 128, :], o[:, cc * 256 : (cc + 1) * 256]
                )

            def emit_A(st):
                j, g = st["j"], st["g"]
                tj = T8[j]
                e, po = st["e"], st["po"]
                for k in range(2):
                    uu = 2 * g + k
                    for cc in range(2):
                        ustop = tj - 2 + cc
                        if uu > ustop:
                            continue
                        nc.tensor.matmul(
                            po[cc][:],
                            e[:, k * 256 + cc * 128 : k * 256 + (cc + 1) * 128],
                            v_sb[:, uu * 257 : (uu + 1) * 257],
                            start=(uu == 0),
                            stop=(uu == ustop),
                        )
                        if uu == ustop:
                            finish_chunk(st, cc)

            # prework: chunk 0 + qT stripe 7 (nothing to overlap with yet)
            for t in proj_tasks(0, 512, [7]):
                t()

            for j in (7, 6, 5, 4, 3, 2, 1, 0):
                tj = T8[j]
                ng = tj // 2
                slot = {
                    "j": j,
                    "qs": qT[:, j * 256 : (j + 1) * 256],
                    "po": [
                        po_pool.tile([128, 257], f32, tag=f"po{cc}", name=f"po{cc}")
                        for cc in range(2)
                    ],
                    "o": opool.tile([128, 512], f32, tag="o", name="o"),
                    "rinv": rpool.tile([128, 2], f32, tag="rinv", name="rinv"),
                }
                # generate this slot's mask window on DVE: m = (c >= t)
                for w in range(4):
                    nc.vector.tensor_scalar(
                        m_sb[:, j * 1024 + w * 256 : j * 1024 + (w + 1) * 256],
                        cif[:],
                        mt_f[:, j * 4 + w : j * 4 + w + 1],
                        None,
                        mybir.AluOpType.is_ge,
                    )
                inject.extend(inject_for.get(j, []))
                # spread remaining inject work over this slot's phases
                npoints = max(1, 2 * ng)
                per_point = -(-len(inject) // npoints)  # ceil
                for g in range(ng):
                    while len(pending) > 4:
                        emit_A(pending.popleft())
                        pop_inject(per_point)
                    st = dict(slot)
                    st["g"] = g
                    emit_S(st)
                    pop_inject(per_point)
                    pending.append(st)
                pop_inject(len(inject))  # flush any leftover proj work
            while pending:
                emit_A(pending.popleft())

    nc.compile()
    return nc


def kernel(encodings_for_q, encodings_for_k, encodings_for_v, mask, Wq, Wk, Wv):
    from concourse.bass_utils import run_bass_kernel_spmd

    if "nc" not in _CACHE:
        _CACHE["nc"] = _build_nc()
    nc = _CACHE["nc"]

    bf = ml_dtypes.bfloat16
    wbase = np.concatenate([Wq.T, Wk.T, Wv.T], axis=1).astype(np.float32)

    if "wa8" not in _CACHE:
        was = []
        for t in range(2):
            R = R_A8 if t == 0 else R_B8
            q0s = _q0s8(R)
            mt = np.zeros((128, 32), dtype=np.float32)
            for j in range(8):
                for w in range(4):
                    # mask[p, c] = (c >= t[p]), t = key - q0
                    mt[:, j * 4 + w] = (
                        (T8[j] - 4 + w) * 128 + np.arange(128) - q0s[j]
                    )
            mt = np.clip(mt, -1, 256)  # exact in bf16
            wa = np.zeros((256, 672), dtype=np.float32)
            wa[:, 0:384] = wbase
            wa[0:128, 384:416] = mt
            was.append(wa)
        _CACHE["wa8"] = was

    in_maps = []
    metas = []
    for c in range(8):
        b, t = c // 2, c % 2
        R = R_A8 if t == 0 else R_B8
        q0s = _q0s8(R)
        eqT = np.concatenate(
            [encodings_for_q[b, q0 : q0 + 256, :].T for q0 in q0s[:7]], axis=1
        )
        wac = _CACHE["wa8"][t].copy()
        wac[:, 416:672] = encodings_for_q[b, q0s[7] : q0s[7] + 256, :].T
        in_maps.append(
            {
                "eq": np.ascontiguousarray(eqT.astype(bf)),
                "ek": np.ascontiguousarray(encodings_for_k[b].T.astype(bf)),
                "ev": np.ascontiguousarray(encodings_for_v[b].T.astype(bf)),
                "wa": np.ascontiguousarray(wac.astype(bf)),
            }
        )
        metas.append((b, q0s))

    res = run_bass_kernel_spmd(nc, in_maps, core_ids=list(range(8)))
    _CACHE["last_res"] = res

    out = np.empty((B, S, DM), dtype=np.float32)
    for c in range(8):
        b, q0s = metas[c]
        oN = res.results[c]["outN"]
        for j, q0 in enumerate(q0s):
            out[b, q0 : q0 + 256, :] = oN[j * 256 : (j + 1) * 256, :]
    return out


# revision 55
# speedup vs baseline: 1.0038x; 1.0038x over previous
"""Causal attention head on 8 trn2 NeuronCores.

Sharding: core c = (batch b = c//2, type t = c%2). Each core handles 8
query stripes of 256 (one slot per stripe) of its batch against the key
prefix each stripe needs. Causal balance: type A gets stripes with nkb
{32,28,24,20,14,10,6,2}, type B {30,26,22,18,16,12,8,4} (136 key-blocks
each). One SPMD program: every core runs the padded template
T8 = {32,28,24,20,16,12,8,4} (144 units); masking and padding are driven
purely by per-core input data (multiplicative bf16 mask tiles), so the
instruction stream is identical on all cores.

Design notes:
  - AV matmul flipped: e (bf16) is the STATIONARY operand, v the moving
    one. v is augmented with a ones column -> rhs [128 keys, 257]; the
    softmax denominator accumulates in psum column 256 for free (no
    separate denominator matmuls; PE matmul cost is proportional to the
    moving-side column count only).
  - Output accumulates as [q, dims] (natural layout). Denominator is a
    per-partition scalar -> reciprocal + broadcast multiply on DVE;
    normalized output DMAs out directly, no host math.
  - Causal early stop per q-chunk: chunk cc of slot j stops its psum
    accumulation at template unit T8[j]-2+cc (beyond-diagonal units are
    masked-zero on every core), then normalizes + ships immediately.
  - Encodings/weights travel as bf16 (quantizing the matmul inputs
    instead of the projection outputs; rel err ~5e-3 vs 3e-3) which
    halves input DMA to ~5.3MB/core.
  - All weights, mask thresholds (clamped to [-1,256], exact in bf16)
    and query stripe 7 ride in one packed input `wa`, so the first
    DMA pair unblocks the whole startup ladder; ek/ev stream in graded
    key chunks (512,1024,1024,1536) matched to slot unlock order.
  - Causal masks are generated on device (gpsimd iota ramp +
    tensor_scalar is_ge against per-core thresholds) instead of DMAed.
  - exp in 2-unit groups ([128,512] activations, 72/core); global lag-4
    software pipeline across slots with kT/qT/v projection tasks
    injected between attention phases to keep PE busy.

Per slot j (query stripe q0..q0+256, tj = T8[j] units):
  sT[u]  = (k[kb=u] @ qT_stripe)      scores transposed [128 keys, 256 q]
  e      = exp(0.125*sT) (*mask)      bf16, groups of 2 units
  po[c] += e[:,c*128:+128].T @ [v[u] | 1]   [128 q, 257] psum, c=0..1
  out[c] = po[c,:256] * (1/po[c,256])       DVE, DMA out [128 q, 256]
"""

import sys

sys.path.insert(0, "/opt/trn_rl_repo")

import numpy as np
import ml_dtypes

B, S, DM, DQ = 4, 4096, 256, 64
T8 = [32, 28, 24, 20, 16, 12, 8, 4]  # padded template: key-blocks per slot
R_A8 = [32, 28, 24, 20, 14, 10, 6, 2]  # real key-blocks, type A cores
R_B8 = [30, 26, 22, 18, 16, 12, 8, 4]  # type B

_CACHE = {}


def _q0s8(R):
    return [(r - 2) * 128 for r in R]


def _build_nc():
    import concourse.bass as bass  # noqa: F401
    import concourse.tile as tile
    from concourse import bacc, mybir

    dt = mybir.dt
    f32, bf = dt.float32, dt.bfloat16
    f32r = dt.float32r

    nc = bacc.Bacc(
        "TRN2",
        target_bir_lowering=False,
        debug=False,
        enable_asserts=False,
        num_devices=8,
    )

    def din(name, shape, d=None):
        d = f32r if d is None else d
        return nc.dram_tensor(name, shape, d, kind="ExternalInput").ap()

    eq = din("eq", [256, 1792], bf)  # stripes 6..0 (stripe 7 rides in wa)
    ek = din("ek", [256, 4096], bf)
    ev = din("ev", [256, 4096], bf)
    # packed [Wq.T | Wk.T | Wv.T | mt | eq stripe 7]; mask thresholds mt
    # (clamped to [-1,256], exact in bf16) live in rows 0:128 of cols
    # 384:416
    wa = din("wa", [256, 672], bf)
    outN = nc.dram_tensor("outN", [2048, 256], f32, kind="ExternalOutput").ap()

    with tile.TileContext(nc) as tc:
        from contextlib import ExitStack

        with ExitStack() as ctx:
            const = ctx.enter_context(tc.tile_pool(name="const", bufs=1))

            # ---- persistent SBUF tensors ----
            eq_sb = [const.tile([128, 1792], bf, tag=f"eq{h}", name=f"eq{h}") for h in range(2)]
            ek_sb = [const.tile([128, 4096], bf, tag=f"ek{h}", name=f"ek{h}") for h in range(2)]
            ev_sb = [const.tile([128, 4096], bf, tag=f"ev{h}", name=f"ev{h}") for h in range(2)]
            wa_sb = const.tile([128, 1344], bf, tag="wa", name="wa")  # 2 x 672

            def wq_s(h):  # [dm_half, 64]
                return wa_sb[:, h * 672 : h * 672 + 64]

            def wk_s(h):
                return wa_sb[:, h * 672 + 64 : h * 672 + 128]

            def wv_s(h):  # [dm_half, 256]
                return wa_sb[:, h * 672 + 128 : h * 672 + 384]

            def eq7_s(h):  # [dm_half, 256] query stripe 7
                return wa_sb[:, h * 672 + 416 : h * 672 + 672]

            mt_sl = wa_sb[:, 384:416]  # thresholds ride in half 0 (bf16)
            mt_f = const.tile([128, 32], dt.float32, tag="mtf", name="mtf")
            m_sb = const.tile([128, 8 * 1024], bf, tag="mi", name="mi")
            cif = const.tile([128, 256], dt.float32, tag="cif", name="cif")
            kT = const.tile([64, 4096], f32r, tag="kT", name="kT")
            qT = const.tile([64, 2048], f32r, tag="qT", name="qT")
            v_sb = const.tile([128, 32 * 257], bf, tag="v", name="v")

            # ones column for every v unit (psum col 256 = softmax denom)
            for t in range(32):
                nc.gpsimd.memset(v_sb[:, t * 257 + 256 : (t + 1) * 257], 1.0)
            # column-index ramp for on-device causal mask generation
            nc.gpsimd.iota(
                cif[:],
                pattern=[[1, 256]],
                base=0,
                channel_multiplier=0,
                allow_small_or_imprecise_dtypes=True,
            )

            # ---- DMAs (SP queue; graded chunks, slot-7-first unlock).
            # With bf16 inputs the stream is issue/DGE-serialization bound,
            # so fewer, consolidated DMAs win.
            def dma_eq(c0, c1):
                for h in range(2):
                    nc.sync.dma_start(
                        eq_sb[h][:, c0:c1], eq[h * 128 : (h + 1) * 128, c0:c1]
                    )

            def dma_kv(t_sb, t_dr, k0, k1, eng=None):
                eng = eng or nc.sync
                for h in range(2):
                    eng.dma_start(
                        t_sb[h][:, k0:k1], t_dr[h * 128 : (h + 1) * 128, k0:k1]
                    )

            for h in range(2):
                nc.sync.dma_start(
                    wa_sb[:, h * 672 : (h + 1) * 672], wa[h * 128 : (h + 1) * 128, :]
                )
            nc.vector.tensor_copy(mt_f[:], mt_sl)  # is_ge needs f32 scalars
            dma_kv(ek_sb, ek, 0, 512)
            dma_kv(ev_sb, ev, 0, 512)
            dma_eq(6 * 256, 7 * 256)  # stripe 6 early (qT6 unblocks slot 6)
            dma_kv(ek_sb, ek, 512, 1024)
            dma_eq(0, 6 * 256)  # stripes 5..0
            dma_kv(ev_sb, ev, 512, 1024)
            for k0, k1 in ((1024, 2048), (2048, 4096)):
                dma_kv(ek_sb, ek, k0, k1)
                dma_kv(ev_sb, ev, k0, k1)

            # ---- pools ----
            ps_pool = ctx.enter_context(tc.tile_pool(name="psc", bufs=2, space="PSUM"))
            pp_pool = ctx.enter_context(tc.tile_pool(name="pp", bufs=2, space="PSUM"))
            po_pool = ctx.enter_context(tc.tile_pool(name="po", bufs=2, space="PSUM"))
            epool = ctx.enter_context(tc.tile_pool(name="e", bufs=6))
            opool = ctx.enter_context(tc.tile_pool(name="o", bufs=2))
            rpool = ctx.enter_context(tc.tile_pool(name="r", bufs=2))

            Exp = mybir.ActivationFunctionType.Exp

            # ---- projection task list (emitted piecemeal between phases) --
            def task_qT(s):
                def run():
                    ps = pp_pool.tile([128, 512], f32, tag="pp", name="pp")
                    for h in range(2):
                        nc.tensor.matmul(
                            ps[0:64, 0:256],
                            wq_s(h),
                            eq7_s(h) if s == 7 else eq_sb[h][:, s * 256 : (s + 1) * 256],
                            start=(h == 0),
                            stop=(h == 1),
                        )
                    nc.vector.tensor_copy(
                        qT[:, s * 256 : (s + 1) * 256], ps[0:64, 0:256]
                    )
                return run

            def task_kT(k0):
                def run():
                    ps = pp_pool.tile([128, 512], f32, tag="pp", name="pp")
                    for h in range(2):
                        nc.tensor.matmul(
                            ps[0:64, 0:512],
                            wk_s(h),
                            ek_sb[h][:, k0 : k0 + 512],
                            start=(h == 0),
                            stop=(h == 1),
                        )
                    nc.vector.tensor_copy(kT[:, k0 : k0 + 512], ps[0:64, 0:512])
                return run

            def task_v(u):
                def run():
                    ps = pp_pool.tile([128, 512], f32, tag="pp", name="pp")
                    for h in range(2):
                        nc.tensor.matmul(
                            ps[:, 0:256],
                            ev_sb[h][:, u * 128 : (u + 1) * 128],
                            wv_s(h),
                            start=(h == 0),
                            stop=(h == 1),
                        )
                    nc.vector.tensor_copy(v_sb[:, u * 257 : u * 257 + 256], ps[:, 0:256])
                return run

            def proj_tasks(k0, k1, qstripes):
                ts_ = [task_qT(s) for s in qstripes]
                ts_ += [task_kT(k) for k in range(k0, k1, 512)]
                ts_ += [task_v(u) for u in range(k0 // 128, k1 // 128)]
                return ts_

            # proj work injected into each slot's phase stream, ordered by
            # deadline and placed no earlier than its chunk's DMA arrival
            inject_for = {
                7: proj_tasks(512, 1024, [6]) + proj_tasks(1024, 1536, [5]),
                6: proj_tasks(1536, 2048, [4])
                + proj_tasks(2048, 2560, [3])
                + proj_tasks(2560, 3072, [2]),
                5: proj_tasks(3072, 4096, [1, 0]),
            }

            # ---- global lag-4 attention pipeline ----
            from collections import deque

            pending = deque()  # AV groups not yet emitted
            inject = deque()  # projection tasks to sprinkle between phases

            def pop_inject(n):
                for _ in range(n):
                    if inject:
                        inject.popleft()()

            def emit_S(st):
                j, g = st["j"], st["g"]
                tj = T8[j]
                ps = ps_pool.tile([128, 512], f32, tag="ps", name="ps")
                for k in range(2):
                    u = 2 * g + k
                    nc.tensor.matmul(
                        ps[:, k * 256 : (k + 1) * 256],
                        kT[:, u * 128 : (u + 1) * 128],
                        st["qs"],
                        start=True,
                        stop=True,
                    )
                e = epool.tile([128, 512], bf, tag="e", name="e")
                nc.scalar.activation(e[:], ps[:], Exp, scale=0.125)
                if 2 * g >= tj - 4:  # group inside the masked last-4 window
                    w0 = 2 * g - (tj - 4)  # window offset in units (0 or 2)
                    msl = slice(j * 1024 + w0 * 256, j * 1024 + (w0 + 2) * 256)
                    nc.vector.tensor_mul(e[:], e[:], m_sb[:, msl])
                st["e"] = e

            def finish_chunk(st, cc):
                j, po = st["j"], st["po"]
                o, rinv = st["o"], st["rinv"]
                nc.vector.reciprocal(rinv[:, cc : cc + 1], po[cc][:, 256:257])
                nc.vector.tensor_scalar_mul(
                    o[:, cc * 256 : (cc + 1) * 256],
                    po[cc][:, 0:256],
                    rinv[:, cc : cc + 1],
                )
                r0 = j * 256 + cc * 128
                nc.sync.dma_start(
                    outN[r0 : r0 + 128, :], o[:, cc * 256 : (cc + 1) * 256]
                )

            def emit_A(st):
                j, g = st["j"], st["g"]
                tj = T8[j]
                e, po = st["e"], st["po"]
                for k in range(2):
                    uu = 2 * g + k
                    for cc in range(2):
                        ustop = tj - 2 + cc
                        if uu > ustop:
                            continue
                        nc.tensor.matmul(
                            po[cc][:],
                            e[:, k * 256 + cc * 128 : k * 256 + (cc + 1) * 128],
                            v_sb[:, uu * 257 : (uu + 1) * 257],
                            start=(uu == 0),
                            stop=(uu == ustop),
                        )
                        if uu == ustop:
                            finish_chunk(st, cc)

            # prework: chunk 0 + qT stripe 7 (nothing to overlap with yet)
            for t in proj_tasks(0, 512, [7]):
                t()

            for j in (7, 6, 5, 4, 3, 2, 1, 0):
                tj = T8[j]
                ng = tj // 2
                slot = {
                    "j": j,
                    "qs": qT[:, j * 256 : (j + 1) * 256],
                    "po": [
                        po_pool.tile([128, 257], f32, tag=f"po{cc}", name=f"po{cc}")
                        for cc in range(2)
                    ],
                    "o": opool.tile([128, 512], f32, tag="o", name="o"),
                    "rinv": rpool.tile([128, 2], f32, tag="rinv", name="rinv"),
                }
                # generate this slot's mask window on DVE: m = (c >= t)
                for w in range(4):
                    nc.vector.tensor_scalar(
                        m_sb[:, j * 1024 + w * 256 : j * 1024 + (w + 1) * 256],
                        cif[:],
                        mt_f[:, j * 4 + w : j * 4 + w + 1],
                        None,
                        mybir.AluOpType.is_ge,
                    )
                inject.extend(inject_for.get(j, []))
                # spread remaining inject work over this slot's phases
                npoints = max(1, 2 * ng)
                per_point = -(-len(inject) // npoints)  # ceil
                for g in range(ng):
                    while len(pending) > 4:
                        emit_A(pending.popleft())
                        pop_inject(per_point)
                    st = dict(slot)
                    st["g"] = g
                    emit_S(st)
                    pop_inject(per_point)
                    pending.append(st)
                pop_inject(len(inject))  # flush any leftover proj work
            while pending:
                emit_A(pending.popleft())

    nc.compile()
    return nc


def kernel(encodings_for_q, encodings_for_k, encodings_for_v, mask, Wq, Wk, Wv):
    from concourse.bass_utils import run_bass_kernel_spmd

    if "nc" not in _CACHE:
        _CACHE["nc"] = _build_nc()
    nc = _CACHE["nc"]

    bf = ml_dtypes.bfloat16
    wbase = np.concatenate([Wq.T, Wk.T, Wv.T], axis=1).astype(np.float32)

    if "wa8" not in _CACHE:
        was = []
        for t in range(2):
            R = R_A8 if t == 0 else R_B8
            q0s = _q0s8(R)
            mt = np.zeros((128, 32), dtype=np.float32)
            for j in range(8):
                for w in range(4):
                    # mask[p, c] = (c >= t[p]), t = key - q0
                    mt[:, j * 4 + w] = (
                        (T8[j] - 4 + w) * 128 + np.arange(128) - q0s[j]
                    )
            mt = np.clip(mt, -1, 256)  # exact in bf16
            wa = np.zeros((256, 672), dtype=np.float32)
            wa[:, 0:384] = wbase
            wa[0:128, 384:416] = mt
            was.append(wa)
        _CACHE["wa8"] = was

    in_maps = []
    metas = []
    for c in range(8):
        b, t = c // 2, c % 2
        R = R_A8 if t == 0 else R_B8
        q0s = _q0s8(R)
        eqT = np.concatenate(
            [encodings_for_q[b, q0 : q0 + 256, :].T for q0 in q0s[:7]], axis=1
        )
        wac = _CACHE["wa8"][t].copy()
        wac[:, 416:672] = encodings_for_q[b, q0s[7] : q0s[7] + 256, :].T
        in_maps.append(
            {
                "eq": np.ascontiguousarray(eqT.astype(bf)),
                "ek": np.ascontiguousarray(encodings_for_k[b].T.astype(bf)),
                "ev": np.ascontiguousarray(encodings_for_v[b].T.astype(bf)),
                "wa": np.ascontiguousarray(wac.astype(bf)),
            }
        )
        metas.append((b, q0s))

    res = run_bass_kernel_spmd(nc, in_maps, core_ids=list(range(8)))
    _CACHE["last_res"] = res

    out = np.empty((B, S, DM), dtype=np.float32)
    for c in range(8):
        b, q0s = metas[c]
        oN = res.results[c]["outN"]
        for j, q0 in enumerate(q0s):
            out[b, q0 : q0 + 256, :] = oN[j * 256 : (j + 1) * 256, :]
    return out


# revision 61
# speedup vs baseline: 1.0061x; 1.0022x over previous
"""Causal attention head on 8 trn2 NeuronCores.

Sharding: core c = (batch b = c//2, type t = c%2). Each core handles 8
query stripes of 256 (one slot per stripe) of its batch against the key
prefix each stripe needs. Causal balance: type A gets stripes with nkb
{32,28,24,20,14,10,6,2}, type B {30,26,22,18,16,12,8,4} (136 key-blocks
each). One SPMD program: every core runs the padded template
T8 = {32,28,24,20,16,12,8,4} (144 units); masking and padding are driven
purely by per-core input data (multiplicative bf16 mask tiles), so the
instruction stream is identical on all cores.

Design notes:
  - AV matmul flipped: e (bf16) is the STATIONARY operand, v the moving
    one. v is augmented with a ones column -> rhs [128 keys, 257]; the
    softmax denominator accumulates in psum column 256 for free (no
    separate denominator matmuls; PE matmul cost is proportional to the
    moving-side column count only).
  - Output accumulates as [q, dims] (natural layout). Denominator is a
    per-partition scalar -> reciprocal + broadcast multiply on DVE;
    normalized output DMAs out directly, no host math.
  - Causal early stop per q-chunk: chunk cc of slot j stops its psum
    accumulation at template unit T8[j]-2+cc (beyond-diagonal units are
    masked-zero on every core), then normalizes + ships immediately.
  - Encodings/weights travel as bf16 (quantizing the matmul inputs
    instead of the projection outputs; rel err ~5e-3 vs 3e-3) which
    halves input DMA to ~5.3MB/core.
  - All weights, mask thresholds (clamped to [-1,256], exact in bf16)
    and query stripe 7 ride in one packed input `wa`, so the first
    DMA pair unblocks the whole startup ladder; ek/ev stream in graded
    key chunks (512,1024,1024,1536) matched to slot unlock order.
  - Causal masks are generated on device (gpsimd iota ramp +
    tensor_scalar is_ge against per-core thresholds) instead of DMAed.
  - exp in 2-unit groups ([128,512] activations, 72/core); global lag-4
    software pipeline across slots with kT/qT/v projection tasks
    injected between attention phases to keep PE busy.

Per slot j (query stripe q0..q0+256, tj = T8[j] units):
  sT[u]  = (k[kb=u] @ qT_stripe)      scores transposed [128 keys, 256 q]
  e      = exp(0.125*sT) (*mask)      bf16, groups of 2 units
  po[c] += e[:,c*128:+128].T @ [v[u] | 1]   [128 q, 257] psum, c=0..1
  out[c] = po[c,:256] * (1/po[c,256])       DVE, DMA out [128 q, 256]
"""

import sys

sys.path.insert(0, "/opt/trn_rl_repo")

import numpy as np
import ml_dtypes

B, S, DM, DQ = 4, 4096, 256, 64
T8 = [32, 28, 24, 20, 16, 12, 8, 4]  # padded template: key-blocks per slot
R_A8 = [32, 28, 24, 20, 14, 10, 6, 2]  # real key-blocks, type A cores
R_B8 = [30, 26, 22, 18, 16, 12, 8, 4]  # type B

_CACHE = {}


def _q0s8(R):
    return [(r - 2) * 128 for r in R]


def _build_nc():
    import concourse.bass as bass  # noqa: F401
    import concourse.tile as tile
    from concourse import bacc, mybir

    dt = mybir.dt
    f32, bf = dt.float32, dt.bfloat16
    f32r = dt.float32r

    nc = bacc.Bacc(
        "TRN2",
        target_bir_lowering=False,
        debug=False,
        enable_asserts=False,
        num_devices=8,
    )

    def din(name, shape, d=None):
        d = f32r if d is None else d
        return nc.dram_tensor(name, shape, d, kind="ExternalInput").ap()

    eq = din("eq", [256, 1792], bf)  # stripes 6..0 (stripe 7 rides in wa)
    ek = din("ek", [256, 4096], bf)
    ev = din("ev", [256, 4096], bf)
    # packed [Wq.T | Wk.T | Wv.T | mt | eq stripe 7]; mask thresholds mt
    # (clamped to [-1,256], exact in bf16) live in rows 0:128 of cols
    # 384:416
    wa = din("wa", [256, 672], bf)
    outN = nc.dram_tensor("outN", [2048, 256], f32, kind="ExternalOutput").ap()

    with tile.TileContext(nc) as tc:
        from contextlib import ExitStack

        with ExitStack() as ctx:
            const = ctx.enter_context(tc.tile_pool(name="const", bufs=1))

            # ---- persistent SBUF tensors ----
            eq_sb = [const.tile([128, 1792], bf, tag=f"eq{h}", name=f"eq{h}") for h in range(2)]
            ek_sb = [const.tile([128, 4096], bf, tag=f"ek{h}", name=f"ek{h}") for h in range(2)]
            ev_sb = [const.tile([128, 4096], bf, tag=f"ev{h}", name=f"ev{h}") for h in range(2)]
            wa_sb = const.tile([128, 1344], bf, tag="wa", name="wa")  # 2 x 672

            def wq_s(h):  # [dm_half, 64]
                return wa_sb[:, h * 672 : h * 672 + 64]

            def wk_s(h):
                return wa_sb[:, h * 672 + 64 : h * 672 + 128]

            def wv_s(h):  # [dm_half, 256]
                return wa_sb[:, h * 672 + 128 : h * 672 + 384]

            def eq7_s(h):  # [dm_half, 256] query stripe 7
                return wa_sb[:, h * 672 + 416 : h * 672 + 672]

            mt_sl = wa_sb[:, 384:416]  # thresholds ride in half 0 (bf16)
            mt_f = const.tile([128, 32], dt.float32, tag="mtf", name="mtf")
            m_sb = const.tile([128, 8 * 1024], bf, tag="mi", name="mi")
            cif = const.tile([128, 256], dt.float32, tag="cif", name="cif")
            kT = const.tile([64, 4096], f32r, tag="kT", name="kT")
            qT = const.tile([64, 2048], f32r, tag="qT", name="qT")
            v_sb = const.tile([128, 32 * 257], bf, tag="v", name="v")

            # ones column for every v unit (psum col 256 = softmax denom)
            for t in range(32):
                nc.gpsimd.memset(v_sb[:, t * 257 + 256 : (t + 1) * 257], 1.0)
            # column-index ramp for on-device causal mask generation
            nc.gpsimd.iota(
                cif[:],
                pattern=[[1, 256]],
                base=0,
                channel_multiplier=0,
                allow_small_or_imprecise_dtypes=True,
            )

            # ---- DMAs (SP queue; graded chunks, slot-7-first unlock).
            # With bf16 inputs the stream is issue/DGE-serialization bound,
            # so fewer, consolidated DMAs win.
            def dma_eq(c0, c1):
                for h in range(2):
                    nc.sync.dma_start(
                        eq_sb[h][:, c0:c1], eq[h * 128 : (h + 1) * 128, c0:c1]
                    )

            def dma_kv(t_sb, t_dr, k0, k1, eng=None):
                eng = eng or nc.sync
                for h in range(2):
                    eng.dma_start(
                        t_sb[h][:, k0:k1], t_dr[h * 128 : (h + 1) * 128, k0:k1]
                    )

            for h in range(2):
                nc.sync.dma_start(
                    wa_sb[:, h * 672 : (h + 1) * 672], wa[h * 128 : (h + 1) * 128, :]
                )
            nc.vector.tensor_copy(mt_f[:], mt_sl)  # is_ge needs f32 scalars
            dma_kv(ek_sb, ek, 0, 512)
            dma_kv(ev_sb, ev, 0, 512)
            dma_eq(6 * 256, 7 * 256)  # stripe 6 early (qT6 unblocks slot 6)
            dma_kv(ek_sb, ek, 512, 1024)
            dma_eq(0, 6 * 256)  # stripes 5..0
            dma_kv(ev_sb, ev, 512, 1024)
            for k0, k1 in ((1024, 2048), (2048, 4096)):
                dma_kv(ek_sb, ek, k0, k1)
                dma_kv(ev_sb, ev, k0, k1)

            # ---- pools ----
            ps_pool = ctx.enter_context(tc.tile_pool(name="psc", bufs=2, space="PSUM"))
            pp_pool = ctx.enter_context(tc.tile_pool(name="pp", bufs=2, space="PSUM"))
            po_pool = ctx.enter_context(tc.tile_pool(name="po", bufs=2, space="PSUM"))
            epool = ctx.enter_context(tc.tile_pool(name="e", bufs=6))
            opool = ctx.enter_context(tc.tile_pool(name="o", bufs=2))
            rpool = ctx.enter_context(tc.tile_pool(name="r", bufs=2))

            Exp = mybir.ActivationFunctionType.Exp

            # ---- projection task list (emitted piecemeal between phases) --
            def task_qT(s):
                def run():
                    ps = pp_pool.tile([128, 512], f32, tag="pp", name="pp")
                    for h in range(2):
                        nc.tensor.matmul(
                            ps[0:64, 0:256],
                            wq_s(h),
                            eq7_s(h) if s == 7 else eq_sb[h][:, s * 256 : (s + 1) * 256],
                            start=(h == 0),
                            stop=(h == 1),
                        )
                    nc.vector.tensor_copy(
                        qT[:, s * 256 : (s + 1) * 256], ps[0:64, 0:256]
                    )
                return run

            def task_kT(k0):
                def run():
                    ps = pp_pool.tile([128, 512], f32, tag="pp", name="pp")
                    for h in range(2):
                        nc.tensor.matmul(
                            ps[0:64, 0:512],
                            wk_s(h),
                            ek_sb[h][:, k0 : k0 + 512],
                            start=(h == 0),
                            stop=(h == 1),
                        )
                    nc.vector.tensor_copy(kT[:, k0 : k0 + 512], ps[0:64, 0:512])
                return run

            def task_v(u):
                def run():
                    ps = pp_pool.tile([128, 512], f32, tag="pp", name="pp")
                    for h in range(2):
                        nc.tensor.matmul(
                            ps[:, 0:256],
                            ev_sb[h][:, u * 128 : (u + 1) * 128],
                            wv_s(h),
                            start=(h == 0),
                            stop=(h == 1),
                        )
                    nc.vector.tensor_copy(v_sb[:, u * 257 : u * 257 + 256], ps[:, 0:256])
                return run

            def proj_tasks(k0, k1, qstripes):
                ts_ = [task_qT(s) for s in qstripes]
                ts_ += [task_kT(k) for k in range(k0, k1, 512)]
                ts_ += [task_v(u) for u in range(k0 // 128, k1 // 128)]
                return ts_

            # proj work injected into each slot's phase stream, ordered by
            # deadline and placed no earlier than its chunk's DMA arrival
            inject_for = {
                7: proj_tasks(512, 1024, [6]) + proj_tasks(1024, 1536, [5]),
                6: proj_tasks(1536, 2048, [4])
                + proj_tasks(2048, 2560, [3])
                + proj_tasks(2560, 3072, [2]),
                5: proj_tasks(3072, 4096, [1, 0]),
            }

            # ---- global lag-4 attention pipeline ----
            from collections import deque

            pending = deque()  # AV groups not yet emitted
            inject = deque()  # projection tasks to sprinkle between phases

            def pop_inject(n):
                for _ in range(n):
                    if inject:
                        inject.popleft()()

            def emit_S(st):
                j, g = st["j"], st["g"]
                tj = T8[j]
                ps = ps_pool.tile([128, 512], f32, tag="ps", name="ps")
                for k in range(2):
                    u = 2 * g + k
                    nc.tensor.matmul(
                        ps[:, k * 256 : (k + 1) * 256],
                        kT[:, u * 128 : (u + 1) * 128],
                        st["qs"],
                        start=True,
                        stop=True,
                    )
                e = epool.tile([128, 512], bf, tag="e", name="e")
                nc.scalar.activation(e[:], ps[:], Exp, scale=0.125)
                if 2 * g >= tj - 4:  # group inside the masked last-4 window
                    w0 = 2 * g - (tj - 4)  # window offset in units (0 or 2)
                    msl = slice(j * 1024 + w0 * 256, j * 1024 + (w0 + 2) * 256)
                    nc.vector.tensor_mul(e[:], e[:], m_sb[:, msl])
                st["e"] = e

            def finish_chunk(st, cc):
                j, po = st["j"], st["po"]
                o, rinv = st["o"], st["rinv"]
                nc.vector.reciprocal(rinv[:, cc : cc + 1], po[cc][:, 256:257])
                nc.vector.tensor_scalar_mul(
                    o[:, cc * 256 : (cc + 1) * 256],
                    po[cc][:, 0:256],
                    rinv[:, cc : cc + 1],
                )
                r0 = j * 256 + cc * 128
                nc.sync.dma_start(
                    outN[r0 : r0 + 128, :], o[:, cc * 256 : (cc + 1) * 256]
                )

            def emit_A(st):
                j, g = st["j"], st["g"]
                tj = T8[j]
                e, po = st["e"], st["po"]
                for k in range(2):
                    uu = 2 * g + k
                    for cc in range(2):
                        ustop = tj - 2 + cc
                        if uu > ustop:
                            continue
                        nc.tensor.matmul(
                            po[cc][:],
                            e[:, k * 256 + cc * 128 : k * 256 + (cc + 1) * 128],
                            v_sb[:, uu * 257 : (uu + 1) * 257],
                            start=(uu == 0),
                            stop=(uu == ustop),
                        )
                        if uu == ustop:
                            finish_chunk(st, cc)

            # prework: chunk 0 + qT stripe 7 (nothing to overlap with yet).
            # psum->sbuf copies ride the Activation engine (idle until the
            # first exp) so DVE mask-gen can't delay the critical ladder.
            Copy = mybir.ActivationFunctionType.Copy
            ps = pp_pool.tile([128, 512], f32, tag="pp", name="pp")
            for h in range(2):
                nc.tensor.matmul(
                    ps[0:64, 0:256], wq_s(h), eq7_s(h),
                    start=(h == 0), stop=(h == 1),
                )
            nc.scalar.activation(qT[:, 7 * 256 : 8 * 256], ps[0:64, 0:256], Copy)
            ps = pp_pool.tile([128, 512], f32, tag="pp", name="pp")
            for h in range(2):
                nc.tensor.matmul(
                    ps[0:64, 0:512], wk_s(h), ek_sb[h][:, 0:512],
                    start=(h == 0), stop=(h == 1),
                )
            nc.scalar.activation(kT[:, 0:512], ps[0:64, 0:512], Copy)
            for t in [task_v(u) for u in range(0, 4)]:
                t()

            for j in (7, 6, 5, 4, 3, 2, 1, 0):
                tj = T8[j]
                ng = tj // 2
                slot = {
                    "j": j,
                    "qs": qT[:, j * 256 : (j + 1) * 256],
                    "po": [
                        po_pool.tile([128, 257], f32, tag=f"po{cc}", name=f"po{cc}")
                        for cc in range(2)
                    ],
                    "o": opool.tile([128, 512], f32, tag="o", name="o"),
                    "rinv": rpool.tile([128, 2], f32, tag="rinv", name="rinv"),
                }
                # generate this slot's mask window on DVE: m = (c >= t)
                for w in range(4):
                    nc.vector.tensor_scalar(
                        m_sb[:, j * 1024 + w * 256 : j * 1024 + (w + 1) * 256],
                        cif[:],
                        mt_f[:, j * 4 + w : j * 4 + w + 1],
                        None,
                        mybir.AluOpType.is_ge,
                    )
                inject.extend(inject_for.get(j, []))
                # spread remaining inject work over this slot's phases
                npoints = max(1, 2 * ng)
                per_point = -(-len(inject) // npoints)  # ceil
                for g in range(ng):
                    while len(pending) > 4:
                        emit_A(pending.popleft())
                        pop_inject(per_point)
                    st = dict(slot)
                    st["g"] = g
                    emit_S(st)
                    pop_inject(per_point)
                    pending.append(st)
                pop_inject(len(inject))  # flush any leftover proj work
            while pending:
                emit_A(pending.popleft())

    nc.compile()
    return nc


def kernel(encodings_for_q, encodings_for_k, encodings_for_v, mask, Wq, Wk, Wv):
    from concourse.bass_utils import run_bass_kernel_spmd

    if "nc" not in _CACHE:
        _CACHE["nc"] = _build_nc()
    nc = _CACHE["nc"]

    bf = ml_dtypes.bfloat16
    wbase = np.concatenate([Wq.T, Wk.T, Wv.T], axis=1).astype(np.float32)

    if "wa8" not in _CACHE:
        was = []
        for t in range(2):
            R = R_A8 if t == 0 else R_B8
            q0s = _q0s8(R)
            mt = np.zeros((128, 32), dtype=np.float32)
            for j in range(8):
                for w in range(4):
                    # mask[p, c] = (c >= t[p]), t = key - q0
                    mt[:, j * 4 + w] = (
                        (T8[j] - 4 + w) * 128 + np.arange(128) - q0s[j]
                    )
            mt = np.clip(mt, -1, 256)  # exact in bf16
            wa = np.zeros((256, 672), dtype=np.float32)
            wa[:, 0:384] = wbase
            wa[0:128, 384:416] = mt
            was.append(wa)
        _CACHE["wa8"] = was

    in_maps = []
    metas = []
    for c in range(8):
        b, t = c // 2, c % 2
        R = R_A8 if t == 0 else R_B8
        q0s = _q0s8(R)
        eqT = np.concatenate(
            [encodings_for_q[b, q0 : q0 + 256, :].T for q0 in q0s[:7]], axis=1
        )
        wac = _CACHE["wa8"][t].copy()
        wac[:, 416:672] = encodings_for_q[b, q0s[7] : q0s[7] + 256, :].T
        in_maps.append(
            {
                "eq": np.ascontiguousarray(eqT.astype(bf)),
                "ek": np.ascontiguousarray(encodings_for_k[b].T.astype(bf)),
                "ev": np.ascontiguousarray(encodings_for_v[b].T.astype(bf)),
                "wa": np.ascontiguousarray(wac.astype(bf)),
            }
        )
        metas.append((b, q0s))

    res = run_bass_kernel_spmd(nc, in_maps, core_ids=list(range(8)))
    _CACHE["last_res"] = res

    out = np.empty((B, S, DM), dtype=np.float32)
    for c in range(8):
        b, q0s = metas[c]
        oN = res.results[c]["outN"]
        for j, q0 in enumerate(q0s):
            out[b, q0 : q0 + 256, :] = oN[j * 256 : (j + 1) * 256, :]
    return out
